# revision 53
# baseline (speedup 1.0000x reference)
"""Trainium2 Bass kernel for nn_DeformableTransformer (6-layer decoder).

Sharding: data-parallel over batch N=16 across 8 NeuronCores (2 batches/core).
On-device layout is feature-major: activations are [D, tokens] with tokens
padded to 304 per batch (TOK=608 per core).  A [F, TOK] tensor with F>128 is
stored as an SBUF tile [128, (F//128)*TOK] (feature tile mt at column block
mt*TOK).

Deformable sampling: per (batch, head) band the 2-tap linear interp becomes a
single gather list r = clip(floor(x),0,tl-1)+start with inner=2 contiguous
elements (taps), gathered by gpsimd.indirect_copy from the bf16 value tensor,
then one fused multiply by slot weights U (replicated via a DRAM bounce) and a
strided grouped reduce on DVE.

Self-contained: hardcodes all shapes; imports only the concourse toolchain.
"""
import math
from contextlib import ExitStack

import numpy as np
import ml_dtypes

import concourse.bass as bass
import concourse.tile as tile
from concourse import bacc, mybir
from concourse import bass_utils

f32 = mybir.dt.float32
f32r = mybir.dt.float32r
bf16 = mybir.dt.bfloat16
u16 = mybir.dt.uint16
i32 = mybir.dt.int32
AF = mybir.ActivationFunctionType
ALU = mybir.AluOpType
AX = mybir.AxisListType

# problem dims
N, LQ, D, H, L, P, DFF, NL = 16, 300, 256, 8, 3, 4, 1024, 6
TLENS = (4096, 2048, 1024)
STARTS = (0, 4096, 6144)
T = 7168
Dh = D // H           # 32
NB = 2                # batches per core
Q = 304               # padded tokens per batch (19*16)
TOK = NB * Q          # 608
TPAD = T + 2
KL = 12 * Q           # 3648 gather indices per band (j-major: k = j*Q + q)
KLH = KL // 2         # half list (j in 0..5 / 6..11)
IDXC = KL // 16       # 228 wrapped columns
NCORES = 8
TEMP = 10000.0


def _f32c(x):
    return np.ascontiguousarray(np.asarray(x, dtype=np.float32))


def _bias2(b):
    """[F] -> [128, F//128] column per M-tile."""
    b = _f32c(b)
    F = b.shape[-1]
    mt = F // 128
    return np.ascontiguousarray(b.reshape(mt, 128).T)


def _host_consts():
    c = {}
    c['ident'] = np.eye(128, dtype=np.float32)
    c['ones_col'] = np.ones((128, 1), np.float32)
    e8 = np.zeros((8, 256), np.float32)
    for h in range(8):
        e8[h, h * 32:(h + 1) * 32] = 1.0
    c['e8_256'] = e8
    g968 = np.zeros((96, 8), np.float32)
    for r in range(96):
        g968[r, r // 12] = 1.0
    c['g96_8'] = g968
    c['e8_96'] = np.ascontiguousarray(g968.T)
    lidx = (np.arange(96) // P) % L
    b396 = np.zeros((3, 96), np.float32)
    for r in range(96):
        b396[lidx[r], r] = 1.0
    b396p = np.zeros((32, 96), np.float32)
    b396p[0:3] = b396
    c['b3_96'] = b396p
    c['b3_96w'] = b396p * (1.0 / (2 * P))
    dim_t = (TEMP ** (2.0 * (np.arange(D) // 2) / D)).astype(np.float32)
    c['invdimt'] = (1.0 / dim_t)[None, :]
    tlr = np.asarray(TLENS, np.float32)[lidx]
    str_ = np.asarray(STARTS, np.float32)[lidx]
    c['rowc'] = np.ascontiguousarray(
        np.stack([tlr, tlr - 1, tlr - 2, str_], 1))     # [96, 4]
    m48 = np.ones((128, 1), np.float32)
    m48[44:48] = 0.0
    c['mask48'] = m48
    return c


def _host_weights(inp):
    """Pre-transposed / permuted weight tensors shared by all cores."""
    w = {}
    # grid W0 permuted for our sine row layout; negated for the
    # sin(2pi*(frac(m)-0.5)) = -sin(2pi*m) range reduction
    gw0 = _f32c(inp['grid_w0'])          # (256, 512)
    W0p = np.zeros((D, 4 * D), np.float32)
    for j in range(2 * D):
        if j < D:
            myrow = j if j % 2 == 0 else D + j
        else:
            jp = j - D
            myrow = 2 * D + jp if jp % 2 == 0 else 3 * D + jp
        W0p[:, myrow] = gw0[:, j]
    W0p = -W0p
    w['w0p_t'] = np.ascontiguousarray(W0p.T)          # [1024, 256] lhsT
    w['grid_b0'] = _bias2(inp['grid_b0'])
    w['grid_w1_t'] = np.ascontiguousarray(_f32c(inp['grid_w1']).T)
    w['grid_b1'] = _bias2(inp['grid_b1'])
    w['qs_w0_t'] = np.ascontiguousarray(_f32c(inp['qs_w0']).T)
    w['qs_b0'] = _bias2(inp['qs_b0'])
    w['qs_w1_t'] = np.ascontiguousarray(_f32c(inp['qs_w1']).T)
    w['qs_b1'] = _bias2(inp['qs_b1'])

    sc = 1.0 / math.sqrt(Dh)
    in_w = _f32c(inp['sa_in_w'])          # [NL, 768, 256]
    in_b = _f32c(inp['sa_in_b'])          # [NL, 768]
    w['saq_t'] = np.ascontiguousarray((in_w[:, :D] * sc).transpose(0, 2, 1))
    w['sak_t'] = np.ascontiguousarray(in_w[:, D:2 * D].transpose(0, 2, 1))
    w['sav_t'] = np.ascontiguousarray(in_w[:, 2 * D:].transpose(0, 2, 1))
    w['saq_b'] = np.stack([_bias2(in_b[l, :D] * sc) for l in range(NL)])
    w['sak_b'] = np.stack([_bias2(in_b[l, D:2 * D]) for l in range(NL)])
    w['sav_b'] = np.stack([_bias2(in_b[l, 2 * D:]) for l in range(NL)])
    w['sao_t'] = np.ascontiguousarray(_f32c(inp['sa_out_w']).transpose(0, 2, 1))
    w['sao_b'] = np.stack([_bias2(_f32c(inp['sa_out_b'])[l]) for l in range(NL)])
    w['offw_t'] = np.ascontiguousarray(_f32c(inp['off_w']).transpose(0, 2, 1))
    w['offb'] = _f32c(inp['off_b'])[:, :, None]              # [NL, 96, 1]
    w['aww_t'] = np.ascontiguousarray(_f32c(inp['aw_w']).transpose(0, 2, 1))
    w['awb'] = _f32c(inp['aw_b'])[:, :, None]
    w['valw_bf'] = np.ascontiguousarray(
        _f32c(inp['val_w']).transpose(0, 2, 1)).astype(ml_dtypes.bfloat16)
    w['valb'] = np.stack([_bias2(_f32c(inp['val_b'])[l]) for l in range(NL)])
    w['outpw_t'] = np.ascontiguousarray(_f32c(inp['outp_w']).transpose(0, 2, 1))
    w['outpb'] = np.stack([_bias2(_f32c(inp['outp_b'])[l]) for l in range(NL)])
    w['f1_t'] = np.ascontiguousarray(_f32c(inp['ffn_w1']).transpose(0, 2, 1))
    w['f1b'] = np.stack([_bias2(_f32c(inp['ffn_b1'])[l]) for l in range(NL)])
    w['f2_bf'] = np.ascontiguousarray(
        _f32c(inp['ffn_w2']).transpose(0, 2, 1)).astype(ml_dtypes.bfloat16)
    w['f2b'] = np.stack([_bias2(_f32c(inp['ffn_b2'])[l]) for l in range(NL)])
    for nm in ('n1', 'n2', 'n3'):
        w[nm + 'g'] = np.stack([_bias2(_f32c(inp[nm + '_g'])[l]) for l in range(NL)])
        w[nm + 'b'] = np.stack([_bias2(_f32c(inp[nm + '_b'])[l]) for l in range(NL)])
    return w


def _host_core_inputs(inp, core):
    b0 = NB * core
    d = {}

    def padT(x):
        F = x.shape[2]
        out = np.zeros((F, TOK), np.float32)
        for n in range(NB):
            out[:, n * Q:n * Q + LQ] = x[n].T
        return out

    d['tgt_t'] = padT(_f32c(inp['tgt'])[b0:b0 + NB])
    seg = padT(_f32c(inp['enc_output_segments'])[b0:b0 + NB])
    d['segc'] = np.ascontiguousarray(seg[0:1])
    d['segw_log'] = np.ascontiguousarray(seg[1:2])
    vr = _f32c(inp['src_valid_ratios'])[b0:b0 + NB]        # [2, 3]
    dur = _f32c(inp['feature_durations'])[b0:b0 + NB]      # [2]
    d['vr_t'] = np.ascontiguousarray(vr.T)
    d['dur_row'] = np.ascontiguousarray(dur[None, :])
    srcm = _f32c(inp['src'])[b0:b0 + NB] * (
        ~np.asarray(inp['src_padding_mask'])[b0:b0 + NB].astype(bool))[..., None]
    d['src_bf'] = np.ascontiguousarray(
        srcm.transpose(0, 2, 1)).astype(ml_dtypes.bfloat16)   # [2, 256, 7168]
    return d


# ---------------------------------------------------------------------------
# device program
# ---------------------------------------------------------------------------


def _build_program():
    nc = bacc.Bacc("TRN2", target_bir_lowering=False, debug=False)
    A = {}

    def din(name, shape, dtype):
        A[name] = nc.dram_tensor(name, list(shape), dtype, kind="ExternalInput").ap()

    din('tgt_t', (D, TOK), f32r)
    din('segc', (1, TOK), f32r)
    din('segw_log', (1, TOK), f32)
    din('vr_t', (3, 2), f32)
    din('dur_row', (1, 2), f32)
    din('src_bf', (NB, D, T), bf16)
    din('ident', (128, 128), f32)
    din('ones_col', (128, 1), f32r)
    din('e8_256', (8, 256), f32r)
    din('g96_8', (96, 8), f32r)
    din('e8_96', (8, 96), f32r)
    din('b3_96', (32, 96), f32r)
    din('b3_96w', (32, 96), f32r)
    din('invdimt', (1, 256), f32r)
    din('rowc', (96, 4), f32)
    din('mask48', (128, 1), f32)
    din('w0p_t', (4 * D, D), f32r)
    din('grid_b0', (128, 2), f32)
    din('grid_w1_t', (D, D), f32r)
    din('grid_b1', (128, 2), f32)
    din('qs_w0_t', (D, D), f32r)
    din('qs_b0', (128, 2), f32)
    din('qs_w1_t', (D, D), f32r)
    din('qs_b1', (128, 2), f32)
    din('saq_t', (NL, D, D), f32r)
    din('sak_t', (NL, D, D), f32r)
    din('sav_t', (NL, D, D), f32r)
    din('saq_b', (NL, 128, 2), f32)
    din('sak_b', (NL, 128, 2), f32)
    din('sav_b', (NL, 128, 2), f32)
    din('sao_t', (NL, D, D), f32r)
    din('sao_b', (NL, 128, 2), f32)
    din('offw_t', (NL, D, 96), f32r)
    din('offb', (NL, 96, 1), f32)
    din('aww_t', (NL, D, 96), f32r)
    din('awb', (NL, 96, 1), f32)
    din('valw_bf', (NL, D, D), bf16)
    din('valb', (NL, 128, 2), f32)
    din('outpw_t', (NL, D, D), f32r)
    din('outpb', (NL, 128, 2), f32)
    din('f1_t', (NL, D, DFF), f32r)
    din('f1b', (NL, 128, 8), f32)
    din('f2_bf', (NL, DFF, D), bf16)
    din('f2b', (NL, 128, 2), f32)
    for nm in ('n1', 'n2', 'n3'):
        din(nm + 'g', (NL, 128, 2), f32)
        din(nm + 'b', (NL, 128, 2), f32)

    out_dram = nc.dram_tensor('out_t', [D, TOK], f32, kind="ExternalOutput").ap()

    with tile.TileContext(nc) as tc:
        _emit(tc, A, out_dram)
    nc.compile()
    return nc


def _emit(tc, A, out_dram):
    nc = tc.nc
    ctx = ExitStack()
    with ctx:
        cpool = ctx.enter_context(tc.tile_pool(name="c", bufs=1))
        wpool = ctx.enter_context(tc.tile_pool(name="w", bufs=1))
        ap1 = ctx.enter_context(tc.tile_pool(name="a1", bufs=1))
        ap3 = ctx.enter_context(tc.tile_pool(name="a3", bufs=3))
        e96p = ctx.enter_context(tc.tile_pool(name="e96", bufs=1))
        pmm = ctx.enter_context(tc.tile_pool(name="pmm", bufs=3, space="PSUM"))
        paux = ctx.enter_context(tc.tile_pool(name="paux", bufs=2, space="PSUM"))
        drp = ctx.enter_context(tc.tile_pool(name="dr", bufs=1, space="DRAM"))
        # gp / vp / srcp are entered after the prologue pool exits (below)

        KLQ = 3 * Q              # quarter gather list (3 j's)
        IDXCQ = KLQ // 16        # 57 wrapped cols per quarter

        # ---- constants ----
        def cload(name, shape, dtype=f32):
            t = cpool.tile(list(shape), dtype, tag=name, name=name + '_sb')
            nc.sync.dma_start(t[:], A[name])
            return t

        ident = cload('ident', (128, 128))
        ones_col = cload('ones_col', (128, 1), dtype=f32r)
        e8_256 = cload('e8_256', (8, 256), dtype=f32r)
        g96_8 = cload('g96_8', (96, 8), dtype=f32r)
        e8_96 = cload('e8_96', (8, 96), dtype=f32r)
        b3_96 = cload('b3_96', (32, 96), dtype=f32r)
        b3_96w = cload('b3_96w', (32, 96), dtype=f32r)
        rowc = cload('rowc', (96, 4))
        mask48 = cload('mask48', (128, 1))
        # prologue-only weights live in a scoped pool freed before the layers
        prop_cm = tc.tile_pool(name="pro", bufs=1)
        prop = prop_cm.__enter__()
        invdimt = prop.tile([1, 256], f32r, tag='invdimt')
        nc.sync.dma_start(invdimt[:], A['invdimt'])
        w0p = prop.tile([128, 8 * D], f32r, tag='w0p')
        for kt in range(8):
            nc.sync.dma_start(w0p[:, kt * D:(kt + 1) * D],
                              A['w0p_t'][kt * 128:(kt + 1) * 128, :])
        grid_b0 = cload('grid_b0', (128, 2))
        gw1 = prop.tile([128, 2 * D], f32r, tag='gw1')
        qw0 = cpool.tile([128, 2 * D], f32r, tag='qw0')
        qw1 = cpool.tile([128, 2 * D], f32r, tag='qw1')
        for kt in range(2):
            nc.sync.dma_start(gw1[:, kt * D:(kt + 1) * D],
                              A['grid_w1_t'][kt * 128:(kt + 1) * 128, :])
            nc.sync.dma_start(qw0[:, kt * D:(kt + 1) * D],
                              A['qs_w0_t'][kt * 128:(kt + 1) * 128, :])
            nc.sync.dma_start(qw1[:, kt * D:(kt + 1) * D],
                              A['qs_w1_t'][kt * 128:(kt + 1) * 128, :])
        grid_b1 = cload('grid_b1', (128, 2))
        qs_b0 = cload('qs_b0', (128, 2))
        qs_b1 = cload('qs_b1', (128, 2))
        zero1 = cpool.tile([128, 1], f32, tag='zero1')
        nc.vector.memset(zero1[:], 0.0)
        eps1 = cpool.tile([1, 1], f32, tag='eps1')
        nc.vector.memset(eps1[:], 1e-5)
        eps128 = cpool.tile([128, 1], f32, tag='eps128')
        nc.vector.memset(eps128[:], 1e-5)
        ones128 = cpool.tile([128, 128], f32, tag='ones128')
        nc.vector.memset(ones128[:], 1.0)

        # flat [row=96, n, q, tap] (1216 elems/row) slot-weight bounce and
        # [row=96, n, rep, p16, c] (1216 elems/row) wrapped-index bounce
        # double-buffered per layer parity: A(l+1,0) writes the other
        # buffer while G(l,1) still reads this layer's (avoids a false
        # write-after-read serialization in DRAM dep tracking)
        u_drams = [drp.tile([96 * 1216], bf16, tag=f'u_dram{k}',
                              name=f'u_dram{k}') for k in range(2)]
        i_drams = [drp.tile([96 * 1216], u16, tag=f'i_dram{k}',
                              name=f'i_dram{k}') for k in range(2)]

        # ---- generic feature-major linear ----
        def fm_linear(dst, wt, bias, x, Mtot, Ktot, func=AF.Identity, ns=(0, 1)):
            kts = Ktot // 128
            mts, msz = (Mtot // 128, 128) if Mtot >= 128 else (1, Mtot)
            for n in ns:
                for mt in range(mts):
                    ps = pmm.tile([128, 512], f32, tag='mm', bufs=3, name='ps_mm')
                    for kt in range(kts):
                        lhs = wt[:, kt * Mtot + mt * msz: kt * Mtot + mt * msz + msz]
                        rhs = x[:, kt * TOK + n * Q: kt * TOK + n * Q + Q]
                        nc.tensor.matmul(ps[0:msz, 0:Q], lhs.bitcast(f32r),
                                         rhs.bitcast(f32r),
                                         start=(kt == 0), stop=(kt == kts - 1))
                    bslice = bias[0:msz, mt:mt + 1]
                    nc.scalar.activation(
                        dst[0:msz, mt * TOK + n * Q: mt * TOK + n * Q + Q],
                        ps[0:msz, 0:Q], func, bias=bslice, scale=1.0)

        # ---- prologue: ref, rin ----
        refc = e96p.tile([1, TOK], f32r, tag='x0f', name='refc')
        nc.sync.dma_start(refc[:], A['segc'])
        segw = e96p.tile([1, TOK], f32, tag='p96a', name='segw')
        nc.sync.dma_start(segw[:], A['segw_log'])
        refw = e96p.tile([1, TOK], f32r, tag='fr', name='refw')
        nc.scalar.activation(refw[:], segw[:], AF.Exp, bias=zero1[0:1, :])

        vrt = cpool.tile([3, 2], f32, tag='vrt')
        nc.sync.dma_start(vrt[:], A['vr_t'])
        durr = cpool.tile([1, 2], f32, tag='durr')
        nc.sync.dma_start(durr[:], A['dur_row'])
        dinv = cpool.tile([1, 2], f32, tag='dinv')
        nc.vector.reciprocal(dinv[:], durr[:])
        dinv3 = cpool.tile([3, 2], f32, tag='dinv3')
        nc.gpsimd.partition_broadcast(dinv3[:], dinv[:])
        svec = cpool.tile([3, 2], f32, tag='svec')
        nc.vector.tensor_mul(svec[:], vrt[:], dinv3[:])

        rin3c = e96p.tile([32, TOK], f32r, tag='ge0', name='rin3c')
        rin3w = e96p.tile([32, TOK], f32r, tag='aw', name='rin3w')
        nc.vector.memset(rin3c[:].bitcast(f32), 0.0)
        nc.vector.memset(rin3w[:].bitcast(f32), 0.0)
        refc3 = e96p.tile([3, TOK], f32, tag='p96b', name='refc3')
        refw3 = e96p.tile([3, TOK], f32, tag='p96c', name='refw3')
        nc.gpsimd.partition_broadcast(refc3[:], refc[:].bitcast(f32))
        nc.gpsimd.partition_broadcast(refw3[:], refw[:].bitcast(f32))
        for n in range(NB):
            sv = svec[:, n:n + 1]
            nc.vector.tensor_scalar_mul(rin3c[0:3, n * Q:(n + 1) * Q],
                                        refc3[:, n * Q:(n + 1) * Q], sv)
            nc.vector.tensor_scalar_mul(rin3w[0:3, n * Q:(n + 1) * Q],
                                        refw3[:, n * Q:(n + 1) * Q], sv)
        rin_c_b = cpool.tile([96, TOK], f32, tag='rin_c_b')
        rin_wh_b = cpool.tile([96, TOK], f32, tag='rin_wh_b')
        for n in range(NB):
            psa = paux.tile([128, Q], f32, tag='aux', bufs=2, name='ps_rin')
            nc.tensor.matmul(psa[0:96, :], b3_96[:].bitcast(f32r),
                             rin3c[:, n * Q:(n + 1) * Q].bitcast(f32r))
            nc.scalar.activation(rin_c_b[:, n * Q:(n + 1) * Q], psa[0:96, :], AF.Copy)
            psb = paux.tile([128, Q], f32, tag='aux', bufs=2, name='ps_rinw')
            nc.tensor.matmul(psb[0:96, :], b3_96w[:].bitcast(f32r),
                             rin3w[:, n * Q:(n + 1) * Q].bitcast(f32r))
            nc.scalar.activation(rin_wh_b[:, n * Q:(n + 1) * Q], psb[0:96, :], AF.Copy)

        # ---- sine embedding + grid MLP, streaming over the 8 K-tiles ----
        # sine k-tile rows (kt = half*4 + trig*2 + mt):
        #   sin slot: sin(2pi*(frac(m)-0.5)); cos slot via +1/4 then re-wrap.
        hacc = ap1.tile([128, 2 * TOK], f32, tag='tA', name='hacc')
        for kt in range(8):
            half, trig, mt = kt // 4, (kt // 2) % 2, kt % 2
            refrow = refc if half == 0 else refw
            skt = prop.tile([128, TOK], f32r, tag='skt', bufs=2, name=f'skt_{kt}')
            for n in range(NB):
                ps = pmm.tile([128, 512], f32, tag='mm', bufs=3, name='ps_sine')
                nc.tensor.matmul(
                    ps[:, 0:Q],
                    invdimt[0:1, mt * 128:(mt + 1) * 128].bitcast(f32r),
                    refrow[0:1, n * Q:(n + 1) * Q].bitcast(f32r))
                m_f = prop.tile([128, Q], f32, tag='mtmp', bufs=4, name='m_f')
                nc.scalar.activation(m_f[:], ps[:, 0:Q], AF.Copy)
                m_i = prop.tile([128, Q], i32, tag='mtmp', bufs=4, name='m_i')
                nc.vector.tensor_copy(m_i[:], m_f[:])
                m_y = prop.tile([128, Q], f32, tag='mtmp', bufs=4, name='m_y')
                nc.vector.tensor_copy(m_y[:], m_i[:])
                m_lt = prop.tile([128, Q], f32, tag='mtmp', bufs=4, name='m_lt')
                nc.vector.tensor_tensor(m_lt[:], m_f[:], m_y[:], ALU.is_lt)
                nc.vector.tensor_sub(m_y[:], m_y[:], m_lt[:])      # floor
                nc.vector.tensor_sub(m_f[:], m_f[:], m_y[:])       # frac
                nc.vector.tensor_scalar(m_f[:], m_f[:], -0.5, None, ALU.add)
                if trig == 1:
                    # g = frac(m)-0.25; cos slot = g - (g >= 0.5)
                    nc.vector.tensor_scalar(m_f[:], m_f[:], 0.25, None, ALU.add)
                    m_w = prop.tile([128, Q], f32, tag='mtmp', bufs=4, name='m_w')
                    nc.vector.tensor_scalar(m_w[:], m_f[:], 0.5, None, ALU.is_ge)
                    nc.vector.tensor_sub(m_f[:], m_f[:], m_w[:])
                nc.scalar.activation(skt[:, n * Q:(n + 1) * Q], m_f[:],
                                     AF.Sin, bias=zero1[:], scale=2 * math.pi)
            for n in range(NB):
                for mo in range(2):
                    ps2 = pmm.tile([128, 512], f32, tag='mm', bufs=3, name='ps_h0')
                    nc.tensor.matmul(
                        ps2[:, 0:Q],
                        w0p[:, kt * D + mo * 128: kt * D + mo * 128 + 128].bitcast(f32r),
                        skt[:, n * Q:(n + 1) * Q].bitcast(f32r))
                    sl = slice(mo * TOK + n * Q, mo * TOK + n * Q + Q)
                    if kt == 0:
                        nc.vector.tensor_copy(hacc[:, sl], ps2[:, 0:Q])
                    else:
                        nc.vector.tensor_add(hacc[:, sl], hacc[:, sl], ps2[:, 0:Q])
        hgrid = ap1.tile([128, 2 * TOK], f32r, tag='tB', name='hgrid')
        for mo in range(2):
            sl = slice(mo * TOK, (mo + 1) * TOK)
            nc.scalar.activation(hgrid[:, sl], hacc[:, sl], AF.Relu,
                                 bias=grid_b0[:, mo:mo + 1], scale=1.0)
        raw_qpos = cpool.tile([128, 2 * TOK], f32, tag='raw_qpos')
        fm_linear(raw_qpos, gw1, grid_b1, hgrid, D, D)
        prop_cm.__exit__(None, None, None)
        gp = ctx.enter_context(tc.tile_pool(name="g", bufs=1))
        vp = ctx.enter_context(tc.tile_pool(name="v", bufs=1))
        srcp = ctx.enter_context(tc.tile_pool(name="s", bufs=1))

        out_cur = ap3.tile([128, 2 * TOK], f32r, tag='res', name='out_l0')
        for mt in range(2):
            nc.sync.dma_start(out_cur[:, mt * TOK:(mt + 1) * TOK],
                              A['tgt_t'][mt * 128:(mt + 1) * 128, :])

        # ---- LN (feature-major; stats over partitions via PE ones-matmul) ----
        # all-ones matmul yields per-token sums REPLICATED on all partitions,
        # so the stats math runs on [128, Q] tiles and no gpsimd broadcast
        # (which competes with the gather queue) is needed.
        def layer_norm(dst, z, g_t, b_t, zsq_tag, t1_tag, n):
            z2 = ap1.tile([128, 2 * TOK], f32r, tag=zsq_tag, name='ln_z2')
            for mt in range(2):
                zsl = slice(mt * TOK + n * Q, mt * TOK + n * Q + Q)
                nc.scalar.activation(z2[:, zsl], z[:, zsl], AF.Square,
                                     bias=zero1[:])
            mus = e96p.tile([128, Q], f32, tag='lnmu', bufs=1, name='ln_mus')
            rstd = e96p.tile([128, Q], f32, tag='lnrstd', bufs=1,
                             name='ln_rstd')
            p1 = paux.tile([128, Q], f32, tag='aux', bufs=2, name='ln_p1')
            p2 = paux.tile([128, Q], f32, tag='aux', bufs=2, name='ln_p2')
            for kt in range(2):
                nc.tensor.matmul(p1[:], ones128[:].bitcast(f32r),
                                 z[:, kt * TOK + n * Q: kt * TOK + n * Q + Q].bitcast(f32r),
                                 start=(kt == 0), stop=(kt == 1))
            for kt in range(2):
                nc.tensor.matmul(p2[:], ones128[:].bitcast(f32r),
                                 z2[:, kt * TOK + n * Q: kt * TOK + n * Q + Q].bitcast(f32r),
                                 start=(kt == 0), stop=(kt == 1))
            nc.vector.tensor_scalar(mus[:], p1[:], 1.0 / D, None, ALU.mult)
            bsq = e96p.tile([128, Q], f32, tag='red', bufs=1, name='ln_bsq')
            nc.vector.tensor_mul(bsq[:], mus[:], mus[:])
            nc.vector.scalar_tensor_tensor(rstd[:], p2[:], 1.0 / D, bsq[:],
                                           ALU.mult, ALU.subtract)
            nc.scalar.activation(rstd[:], rstd[:], AF.Sqrt, bias=eps128[:])
            nc.vector.reciprocal(rstd[:], rstd[:])
            t1 = ap1.tile([128, 2 * TOK], f32, tag=t1_tag, name='ln_t1')
            for mt in range(2):
                sl = slice(mt * TOK + n * Q, mt * TOK + n * Q + Q)
                nc.vector.tensor_sub(t1[:, sl], z[:, sl], mus[:])
                nc.vector.tensor_mul(t1[:, sl], t1[:, sl], rstd[:])
                nc.vector.scalar_tensor_tensor(
                    dst[:, sl], t1[:, sl], g_t[:, mt:mt + 1],
                    b_t[:, mt:mt + 1].to_broadcast((128, Q)),
                    ALU.mult, ALU.add)

        # ---- per-layer weight loader ----
        def wload(name, l, shape, dtype=f32, bufs=1):
            t = wpool.tile(list(shape), dtype, tag=name, bufs=bufs,
                           name=f'{name}_{l}')
            src = A[name][l]
            if shape[1] != src.shape[1]:
                kts = src.shape[0] // 128
                Mm = src.shape[1]
                for kt in range(kts):
                    nc.sync.dma_start(t[:, kt * Mm:(kt + 1) * Mm],
                                      src[kt * 128:(kt + 1) * 128, :])
            else:
                nc.sync.dma_start(t[0:shape[0], :], src)
            return t

        ksz = (128, 128, 48)

        # ================= layers =================
        # Per-batch phase split: A(n) = qpos/SA/LN1/off-aw/sampling/bounce/
        # val-proj, G(n) = gather+reduce, B(n) = outp/LN2/FFN/LN3.  Emission
        # order A0 A1 G0 G1 B0 B1 keeps batch-1 compute underneath batch-0's
        # serial gpsimd gather stream (the two batches are independent).
        cur = [None]

        def make_phases(l):
            st = {}
            u_dram = u_drams[l % 2]
            i_dram = i_drams[l % 2]

            def phase_A(n):
                if n == 0:
                    st['saq_t'] = wload('saq_t', l, (128, 2 * D), dtype=f32r)
                    st['sak_t'] = wload('sak_t', l, (128, 2 * D), dtype=f32r)
                    st['sav_t'] = wload('sav_t', l, (128, 2 * D), dtype=f32r)
                    st['sao_t'] = wload('sao_t', l, (128, 2 * D), dtype=f32r)
                    st['saq_b'] = wload('saq_b', l, (128, 2))
                    st['sak_b'] = wload('sak_b', l, (128, 2))
                    st['sav_b'] = wload('sav_b', l, (128, 2))
                    st['sao_b'] = wload('sao_b', l, (128, 2))
                    st['offw_t'] = wload('offw_t', l, (128, 2 * 96), dtype=f32r)
                    st['offb'] = wload('offb', l, (96, 1))
                    st['aww_t'] = wload('aww_t', l, (128, 2 * 96), dtype=f32r)
                    st['awb'] = wload('awb', l, (96, 1))
                    st['valw'] = wload('valw_bf', l, (128, 2 * D), dtype=bf16)
                    st['valb'] = wload('valb', l, (128, 2))
                    st['n1g'] = wload('n1g', l, (128, 2))
                    st['n1b'] = wload('n1b', l, (128, 2))
                    st['out_cur'] = cur[0]
                    st['qpos'] = ap1.tile([128, 2 * TOK], f32, tag='qpos',
                                          name=f'qpos_{l}')
                    st['out_ln1'] = ap3.tile([128, 2 * TOK], f32r, tag='res',
                                             name=f'oln1_{l}')

                saq_t = st['saq_t']; sak_t = st['sak_t']; sav_t = st['sav_t']
                sao_t = st['sao_t']; saq_b = st['saq_b']; sak_b = st['sak_b']
                sav_b = st['sav_b']; sao_b = st['sao_b']
                offw_t = st['offw_t']; offb = st['offb']
                aww_t = st['aww_t']; awb = st['awb']
                valw = st['valw']; valb = st['valb']
                out_cur = st['out_cur']; qpos = st['qpos']
                out_ln1 = st['out_ln1']
                # per-batch val tile: A(l+1,0) then only WARs on G(l,0)
                st[f'val{n}'] = vp.tile([128, 2, TPAD // 2, 2], bf16,
                                        tag=f'val{n}', bufs=1,
                                        name=f'val_{l}_{n}')
                valboth = st[f'val{n}']
                ql = slice(n * Q, (n + 1) * Q)
                hq = ap1.tile([128, 2 * TOK], f32r, tag='tA', name=f'hq_{l}_{n}')
                fm_linear(hq, qw0, qs_b0, out_cur, D, D, func=AF.Relu, ns=[n])
                fm_linear(qpos, qw1, qs_b1, hq, D, D, ns=[n])
                qsa = ap1.tile([128, 2 * TOK], f32r, tag='tB',
                               name=f'qsa_{l}_{n}')
                for mt in range(2):
                    sl = slice(mt * TOK + n * Q, mt * TOK + n * Q + Q)
                    nc.vector.tensor_mul(qpos[:, sl], qpos[:, sl],
                                         raw_qpos[:, sl])
                    nc.vector.tensor_add(qsa[:, sl], out_cur[:, sl],
                                         qpos[:, sl])

                # ---- self attention (batch n) ----
                qh_p = ap1.tile([96, 3 * TOK], f32r, tag='qh', name=f'qh_{l}_{n}')
                kh_p = ap1.tile([96, 3 * TOK], f32r, tag='kh', name=f'kh_{l}_{n}')
                vh_t = ap1.tile([128, 2 * TOK], f32, tag='tC', name=f'vh_{l}_{n}')

                def qk_proj(dst_h, wt, bias):
                    for mt in range(2):
                        ps = pmm.tile([128, 512], f32, tag='mm', bufs=3,
                                      name='ps_qk')
                        for kt in range(2):
                            nc.tensor.matmul(
                                ps[:, 0:Q],
                                wt[:, kt * D + mt * 128: kt * D + mt * 128 + 128].bitcast(f32r),
                                qsa[:, kt * TOK + n * Q: kt * TOK + n * Q + Q].bitcast(f32r),
                                start=(kt == 0), stop=(kt == 1))
                        for hq4 in range(4):
                            h = mt * 4 + hq4
                            pb, cb = (h % 3) * 32, (h // 3) * TOK
                            nc.scalar.activation(
                                dst_h[pb:pb + 32, cb + n * Q: cb + n * Q + Q],
                                ps[hq4 * 32:(hq4 + 1) * 32, 0:Q], AF.Identity,
                                bias=bias[hq4 * 32:(hq4 + 1) * 32, mt:mt + 1],
                                scale=1.0)

                qk_proj(qh_p, saq_t, saq_b)
                qk_proj(kh_p, sak_t, sak_b)
                fm_linear(vh_t, sav_t, sav_b, out_cur, D, D, ns=[n])

                OA = ap1.tile([128, 2 * TOK], f32r, tag='tA', name=f'OA_{l}_{n}')
                Scol = e96p.tile([8, TOK], f32, tag='w96b', name=f'Scol_{l}_{n}')
                vh_tok = ap1.tile([128, 3 * D], f32r, tag='vh_tok',
                                  name=f'vhtok_{l}_{n}')
                for ft in range(2):
                    for kt in range(3):
                        kn = ksz[kt]
                        pt = paux.tile([128, 128], f32, tag='tr', bufs=1,
                                       name='ps_tr')
                        nc.tensor.transpose(
                            pt[0:kn, 0:128],
                            vh_t[:, ft * TOK + n * Q + kt * 128:
                                 ft * TOK + n * Q + kt * 128 + kn],
                            ident[:])
                        nc.scalar.activation(
                            vh_tok[0:kn, kt * D + ft * 128: kt * D + ft * 128 + 128],
                            pt[0:kn, 0:128], AF.Copy)
                for h in range(8):
                    pb, cb = (h % 3) * 32, (h // 3) * TOK
                    mtq = h // 4
                    pq = (h % 4) * 32
                    qh_sl = qh_p[pb:pb + 32, cb + n * Q: cb + n * Q + Q]
                    e_sb = ap3.tile([128, 3 * Q], f32r, tag='e_sb', bufs=1,
                                    name=f'esb_{l}_{n}_{h}')
                    for kt in range(3):
                        kn = ksz[kt]
                        ps = pmm.tile([128, 512], f32, tag='mm', bufs=3,
                                      name='ps_sc')
                        nc.tensor.matmul(
                            ps[0:kn, 0:Q],
                            kh_p[pb:pb + 32, cb + n * Q + kt * 128:
                                 cb + n * Q + kt * 128 + kn].bitcast(f32r),
                            qh_sl[:].bitcast(f32r))
                        nc.scalar.activation(e_sb[0:kn, kt * Q:(kt + 1) * Q],
                                             ps[0:kn, 0:Q], AF.Exp,
                                             bias=zero1[0:kn, :])
                    nc.vector.tensor_scalar_mul(e_sb[0:48, 2 * Q:3 * Q],
                                                e_sb[0:48, 2 * Q:3 * Q],
                                                mask48[0:48, :])
                    poa = paux.tile([128, Q], f32, tag='aux', bufs=2,
                                    name='ps_oa')
                    psum_s = paux.tile([8, Q], f32, tag='scol', bufs=1,
                                       name='ps_s')
                    for kt in range(3):
                        kn = ksz[kt]
                        nc.tensor.matmul(
                            poa[0:32, :],
                            vh_tok[0:kn, kt * D + h * 32: kt * D + h * 32 + 32].bitcast(f32r),
                            e_sb[0:kn, kt * Q:(kt + 1) * Q].bitcast(f32r),
                            start=(kt == 0), stop=(kt == 2))
                    for kt in range(3):
                        kn = ksz[kt]
                        nc.tensor.matmul(
                            psum_s[0:1, :],
                            ones_col[0:kn, :].bitcast(f32r),
                            e_sb[0:kn, kt * Q:(kt + 1) * Q].bitcast(f32r),
                            start=(kt == 0), stop=(kt == 2))
                    nc.scalar.activation(
                        OA[pq:pq + 32, mtq * TOK + n * Q: mtq * TOK + n * Q + Q],
                        poa[0:32, :], AF.Copy)
                    stmp = e96p.tile([1, Q], f32, tag='stmp', bufs=1,
                                     name=f'stmp_{l}_{n}_{h}')
                    nc.vector.tensor_copy(stmp[:], psum_s[0:1, :])
                    nc.sync.dma_start(Scol[h:h + 1, n * Q:(n + 1) * Q], stmp[:])

                Rrec = e96p.tile([8, TOK], f32r, tag='w96c', name=f'Rrec_{l}_{n}')
                with nc.allow_low_precision(reason="f32r rounding of recip"):
                    nc.vector.reciprocal(Rrec[:, ql], Scol[:, ql])
                for mt in range(2):
                    pr = paux.tile([128, Q], f32, tag='aux', bufs=2, name='ps_r')
                    nc.tensor.matmul(pr[:],
                                     e8_256[:, mt * 128:(mt + 1) * 128].bitcast(f32r),
                                     Rrec[:, ql].bitcast(f32r))
                    sl = slice(mt * TOK + n * Q, mt * TOK + n * Q + Q)
                    nc.vector.tensor_mul(OA[:, sl], OA[:, sl], pr[:])
                z1 = ap1.tile([128, 2 * TOK], f32r, tag='tB', name=f'z1_{l}_{n}')
                for mt in range(2):
                    ps = pmm.tile([128, 512], f32, tag='mm', bufs=3,
                                  name='ps_sao')
                    for kt in range(2):
                        nc.tensor.matmul(
                            ps[:, 0:Q],
                            sao_t[:, kt * D + mt * 128: kt * D + mt * 128 + 128].bitcast(f32r),
                            OA[:, kt * TOK + n * Q: kt * TOK + n * Q + Q].bitcast(f32r),
                            start=(kt == 0), stop=(kt == 1))
                    sl = slice(mt * TOK + n * Q, mt * TOK + n * Q + Q)
                    nc.vector.scalar_tensor_tensor(
                        z1[:, sl], ps[:, 0:Q], sao_b[:, mt:mt + 1],
                        out_cur[:, sl], ALU.add, ALU.add)
                layer_norm(out_ln1, z1, st['n1g'], st['n1b'], 'tC', 'tA', n)

                # ---- deform offsets / sampling (batch n) ----
                xca = ap1.tile([128, 2 * TOK], f32r, tag='tA', name=f'xca_{l}_{n}')
                for mt in range(2):
                    sl = slice(mt * TOK + n * Q, mt * TOK + n * Q + Q)
                    nc.vector.tensor_add(xca[:, sl], out_ln1[:, sl],
                                         qpos[:, sl])
                offT = e96p.tile([96, TOK], f32, tag='p96a', name=f'offT_{l}_{n}')
                awT = e96p.tile([96, TOK], f32, tag='p96b', name=f'awT_{l}_{n}')
                fm_linear(offT, offw_t, offb, xca, 96, D, ns=[n])
                fm_linear(awT, aww_t, awb, xca, 96, D, ns=[n])
                eaw = e96p.tile([96, TOK], f32r, tag='p96c', name=f'eaw_{l}_{n}')
                nc.scalar.activation(eaw[:, ql], awT[:, ql], AF.Exp,
                                     bias=zero1[0:96, :])
                aw = e96p.tile([96, TOK], f32, tag='aw', name=f'aw_{l}_{n}')
                pss = paux.tile([8, Q], f32, tag='scol', bufs=1, name='ps_aws')
                nc.tensor.matmul(pss[:], g96_8[:].bitcast(f32r),
                                 eaw[:, ql].bitcast(f32r))
                srec = e96p.tile([8, Q], f32r, tag='srec', bufs=1,
                                 name='aw_srec')
                with nc.allow_low_precision(reason="f32r rounding of aw recip"):
                    nc.vector.reciprocal(srec[:], pss[:])
                pre = paux.tile([128, Q], f32, tag='aux', bufs=2, name='ps_awr')
                nc.tensor.matmul(pre[0:96, :], e8_96[:].bitcast(f32r),
                                 srec[:].bitcast(f32r))
                nc.vector.tensor_mul(aw[:, ql], eaw[:, ql], pre[0:96, :])
                xx = e96p.tile([96, TOK], f32, tag='p96b', name=f'xx_{l}_{n}')
                nc.vector.tensor_mul(xx[:, ql], offT[:, ql], rin_wh_b[:, ql])
                nc.vector.tensor_add(xx[:, ql], xx[:, ql], rin_c_b[:, ql])
                nc.vector.tensor_scalar(xx[:, ql], xx[:, ql], rowc[:, 0:1],
                                        -0.5, ALU.mult, ALU.add)
                x_i = e96p.tile([96, TOK], i32, tag='p96c', name=f'xi_{l}_{n}')
                nc.vector.tensor_copy(x_i[:, ql], xx[:, ql])
                x0f = e96p.tile([96, TOK], f32, tag='x0f', name=f'x0f_{l}_{n}')
                nc.vector.tensor_copy(x0f[:, ql], x_i[:, ql])
                xlt = e96p.tile([96, TOK], f32, tag='w96a', name=f'xlt_{l}_{n}')
                nc.vector.tensor_tensor(xlt[:, ql], xx[:, ql], x0f[:, ql],
                                        ALU.is_lt)
                nc.vector.tensor_sub(x0f[:, ql], x0f[:, ql], xlt[:, ql])
                fr = e96p.tile([96, TOK], f32, tag='fr', name=f'fr_{l}_{n}')
                nc.vector.tensor_sub(fr[:, ql], xx[:, ql], x0f[:, ql])
                rf = e96p.tile([96, TOK], f32, tag='w96a', name=f'rf_{l}_{n}')
                nc.vector.tensor_scalar(rf[:, ql], x0f[:, ql], 0.0,
                                        rowc[:, 1:2], ALU.max, ALU.min)
                nc.vector.tensor_scalar(rf[:, ql], rf[:, ql], rowc[:, 3:4],
                                        None, ALU.add)
                r16 = e96p.tile([96, TOK], u16, tag='r16', name=f'r16_{l}_{n}')
                nc.vector.tensor_copy(r16[:, ql], rf[:, ql])
                ge0 = e96p.tile([96, TOK], f32, tag='ge0', name=f'ge0_{l}_{n}')
                nc.vector.tensor_scalar(ge0[:, ql], x0f[:, ql], 0.0, None,
                                        ALU.is_ge)
                le1 = e96p.tile([96, TOK], f32, tag='p96a', name=f'le1_{l}_{n}')
                nc.vector.tensor_scalar(le1[:, ql], x0f[:, ql], rowc[:, 1:2],
                                        None, ALU.is_le)
                Uint = e96p.tile([96, TOK, 2], bf16, tag='u01', name=f'u01_{l}_{n}')
                t0 = e96p.tile([96, TOK], f32, tag='w96c', name=f't0_{l}_{n}')
                nc.vector.tensor_scalar(t0[:, ql], fr[:, ql], -1.0, 1.0,
                                        ALU.mult, ALU.add)
                nc.vector.tensor_mul(t0[:, ql], t0[:, ql], ge0[:, ql])
                nc.vector.tensor_mul(t0[:, ql], t0[:, ql], le1[:, ql])
                tm1 = e96p.tile([96, TOK], f32, tag='w96b', name=f'tm1_{l}_{n}')
                nc.vector.tensor_scalar(tm1[:, ql], x0f[:, ql], -1.0, None,
                                        ALU.is_equal)
                nc.vector.tensor_mul(tm1[:, ql], tm1[:, ql], fr[:, ql])
                nc.vector.tensor_add(t0[:, ql], t0[:, ql], tm1[:, ql])
                nc.vector.tensor_mul(
                    Uint[:, ql, 0:1].rearrange("p q t -> p (q t)"),
                    t0[:, ql], aw[:, ql])
                le2 = e96p.tile([96, TOK], f32, tag='p96b', name=f'le2_{l}_{n}')
                nc.vector.tensor_scalar(le2[:, ql], x0f[:, ql], rowc[:, 2:3],
                                        None, ALU.is_le)
                nc.vector.tensor_mul(le2[:, ql], le2[:, ql], ge0[:, ql])
                nc.vector.tensor_mul(le2[:, ql], le2[:, ql], fr[:, ql])
                nc.vector.tensor_mul(
                    Uint[:, ql, 1:2].rearrange("p q t -> p (q t)"),
                    le2[:, ql], aw[:, ql])
                nc.vector.memset(
                    Uint[:, n * Q + LQ:(n + 1) * Q, :]
                    .rearrange("p q t -> p (q t)"), 0.0)
                nc.vector.memset(r16[:, n * Q + LQ:(n + 1) * Q], 0)
                r16w = e96p.tile([96, TOK], u16, tag='r16w', name=f'r16w_{l}_{n}')
                nc.vector.tensor_copy(
                    r16w[:, ql].rearrange("p (s c) -> p s c", s=16, c=19),
                    r16[:, ql].rearrange("p (c s) -> p s c", c=19, s=16))
                # bounce (batch-n halves of the flat layouts)
                nc.scalar.dma_start(
                    bass.AP(tensor=u_dram.tensor,
                            offset=u_dram[:].offset + n * 608,
                            ap=[[1216, 96], [1, 608]]),
                    Uint[:, ql, :].rearrange("p q t -> p (q t)"))
                for rep in range(2):
                    nc.scalar.dma_start(
                        bass.AP(tensor=i_dram.tensor,
                                offset=i_dram[:].offset + n * 608 + rep * 304,
                                ap=[[1216, 96], [1, 304]]),
                        r16w[:, ql])
                # val projection (both head-quads) from chunked src
                for hqd in range(2):
                    nc.vector.memset(
                        valboth[:, hqd]
                        .rearrange("p a b -> p (a b)")[:, T:TPAD], 0.0)
                for ch in range(14):
                    xs = srcp.tile([128, 2 * 512], bf16, tag='xs', bufs=1,
                                   name='src_chunk')
                    src_ap = bass.AP(
                        tensor=A['src_bf'].tensor,
                        offset=A['src_bf'].offset + n * D * T + ch * 512,
                        ap=[[T, 128], [128 * T, 2], [1, 512]])
                    nc.scalar.dma_start(xs[:], src_ap)
                    for hqd in range(2):
                        ps = pmm.tile([128, 512], f32, tag='mm', bufs=3,
                                      name='ps_val')
                        for kt in range(2):
                            nc.tensor.matmul(
                                ps[:],
                                valw[:, kt * D + hqd * 128: kt * D + hqd * 128 + 128],
                                xs[:, kt * 512:(kt + 1) * 512],
                                start=(kt == 0), stop=(kt == 1))
                        nc.scalar.activation(
                            valboth[:, hqd]
                            .rearrange("p a b -> p (a b)")
                            [:, ch * 512:(ch + 1) * 512],
                            ps[:], AF.Identity,
                            bias=valb[:, hqd:hqd + 1], scale=1.0)
                if n == 1:
                    # stash out_ln1 batch-1 cols: B(l,1) runs after A(l+1,0)
                    # has recycled the 'res' ring, so it must not read out_ln1
                    ob = e96p.tile([128, 2 * Q], f32, tag='oln1b', bufs=1,
                                   name=f'oln1b_{l}')
                    for mt in range(2):
                        nc.vector.tensor_copy(
                            ob[:, mt * Q:(mt + 1) * Q],
                            out_ln1[:, mt * TOK + Q: mt * TOK + 2 * Q])
                    st['oln1b'] = ob

            def load_idxt(n, hqd, nm):
                idxt = gp.tile([128, 232], u16, tag='idxt', bufs=2,
                               name=f'{nm}_{l}_{n}_{hqd}')
                for b4 in range(4):
                    for qd in range(4):
                        row0 = (hqd * 4 + b4) * 12 + qd * 3
                        srcbi = bass.AP(
                            tensor=i_dram.tensor,
                            offset=i_dram[:].offset + row0 * 1216 + n * 608,
                            ap=[[19, 32], [1216, 3], [1, 19]])
                        dstbi = idxt[b4 * 32:(b4 + 1) * 32,
                                     qd * 58:qd * 58 + 57]
                        nc.sync.dma_start(
                            dstbi.rearrange("p (j c) -> p j c", j=3), srcbi)
                return idxt

            def phase_G_pre(n):
                # issue batch-n gather-index reads while the sync queue is
                # still clean (before FFN-weight / src-chunk issue stalls)
                for hqd in range(2):
                    st[f'idxt_{n}_{hqd}'] = load_idxt(n, hqd, 'idxp')

            def phase_G(n):
                if n == 0:
                    st['OD'] = ap1.tile([128, 2 * TOK], f32r, tag='od',
                                        name=f'OD_{l}')
                valboth = st[f'val{n}']
                OD = st['OD']
                for hqd in range(2):
                    val = valboth[:, hqd]
                    idxt = st.pop(f'idxt_{n}_{hqd}', None)
                    if idxt is None:
                        idxt = load_idxt(n, hqd, 'idx')
                    osl = slice(hqd * TOK + n * Q, hqd * TOK + n * Q + Q)
                    for qd in range(4):
                        Ut = gp.tile([128, KLQ, 2], bf16, tag='Ut', bufs=1,
                                     name=f'U_{l}_{n}_{hqd}_{qd}')
                        for b4 in range(4):
                            row0 = (hqd * 4 + b4) * 12 + qd * 3
                            srcb = bass.AP(
                                tensor=u_dram.tensor,
                                offset=u_dram[:].offset + row0 * 1216 + n * 608,
                                ap=[[0, 32], [1216, 3], [1, 608]])
                            nc.sync.dma_start(
                                Ut[b4 * 32:(b4 + 1) * 32]
                                .rearrange("p a b -> p (a b)")
                                .rearrange("p (j e) -> p j e", j=3),
                                srcb)
                        gt = gp.tile([128, KLQ, 2], bf16, tag='gt', bufs=2,
                                     name=f'g_{l}_{n}_{hqd}_{qd}')
                        # ISA caps indirect_copy at 512 output positions
                        nc.gpsimd.indirect_copy(
                            gt[:, 0:512, :], val,
                            idxt[:, qd * 58: qd * 58 + 32], True)
                        nc.gpsimd.indirect_copy(
                            gt[:, 512:912, :], val,
                            idxt[:, qd * 58 + 32: qd * 58 + 57], True)
                        nc.vector.tensor_mul(gt[:], gt[:], Ut[:])
                        red = e96p.tile([128, Q], f32, tag='red', bufs=1,
                                        name=f'red_{l}_{n}_{hqd}_{qd}')
                        nc.vector.tensor_reduce(
                            red[:],
                            gt[:].rearrange("p (j q) t -> p q j t", j=3, q=Q),
                            AX.XY, ALU.add)
                        if qd == 0:
                            nc.vector.tensor_copy(OD[:, osl], red[:])
                        else:
                            nc.vector.tensor_add(OD[:, osl], OD[:, osl],
                                                 red[:])

            def phase_B(n):
                if n == 0:
                    st['outpw_t'] = wload('outpw_t', l, (128, 2 * D),
                                          dtype=f32r)
                    st['outpb'] = wload('outpb', l, (128, 2))
                    st['f1b'] = wload('f1b', l, (128, 8))
                    st['f2b'] = wload('f2b', l, (128, 2))
                    for nm in ('n2', 'n3'):
                        st[nm + 'g'] = wload(nm + 'g', l, (128, 2))
                        st[nm + 'b'] = wload(nm + 'b', l, (128, 2))
                    st['out_ln2'] = ap3.tile([128, 2 * TOK], f32r, tag='res',
                                             name=f'oln2_{l}')
                    st['out_new'] = ap3.tile([128, 2 * TOK], f32r, tag='res',
                                             name=f'out_{l + 1}')
                    cur[0] = st['out_new']
                outpw_t = st['outpw_t']; outpb = st['outpb']
                f1b = st['f1b']; f2b = st['f2b']
                out_ln2 = st['out_ln2']; out_new = st['out_new']
                OD = st['OD']
                z2t = ap1.tile([128, 2 * TOK], f32r, tag='tA',
                               name=f'z2t_{l}_{n}')
                for mt in range(2):
                    ps = pmm.tile([128, 512], f32, tag='mm', bufs=3,
                                  name='ps_op')
                    for kt in range(2):
                        nc.tensor.matmul(
                            ps[:, 0:Q],
                            outpw_t[:, kt * D + mt * 128: kt * D + mt * 128 + 128].bitcast(f32r),
                            OD[:, kt * TOK + n * Q: kt * TOK + n * Q + Q].bitcast(f32r),
                            start=(kt == 0), stop=(kt == 1))
                    sl = slice(mt * TOK + n * Q, mt * TOK + n * Q + Q)
                    res1 = (st['out_ln1'][:, sl] if n == 0 else
                            st['oln1b'][:, mt * Q:(mt + 1) * Q])
                    nc.vector.scalar_tensor_tensor(
                        z2t[:, sl], ps[:, 0:Q], outpb[:, mt:mt + 1],
                        res1, ALU.add, ALU.add)
                layer_norm(out_ln2, z2t, st['n2g'], st['n2b'], 'tC', 'tB', n)

                z3 = ap1.tile([128, 2 * TOK], f32r, tag='tA', name=f'z3_{l}_{n}')
                for half in range(2):
                    h1h = ap1.tile([128, 4 * TOK], bf16, tag='h1h',
                                   name=f'h1_{l}_{n}_{half}')
                    for mtl in range(4):
                        mt = half * 4 + mtl
                        f1sub = wpool.tile([128, 2 * 128], f32r, tag='f1sub',
                                           bufs=2, name=f'f1s_{l}_{n}_{mt}')
                        for kt in range(2):
                            nc.sync.dma_start(
                                f1sub[:, kt * 128:(kt + 1) * 128],
                                A['f1_t'][l][kt * 128:(kt + 1) * 128,
                                             mt * 128:(mt + 1) * 128])
                        ps = pmm.tile([128, 512], f32, tag='mm', bufs=3,
                                      name='ps_f1')
                        for kt in range(2):
                            nc.tensor.matmul(
                                ps[:, 0:Q],
                                f1sub[:, kt * 128:(kt + 1) * 128].bitcast(f32r),
                                out_ln2[:, kt * TOK + n * Q: kt * TOK + n * Q + Q].bitcast(f32r),
                                start=(kt == 0), stop=(kt == 1))
                        nc.scalar.activation(
                            h1h[:, mtl * TOK + n * Q: mtl * TOK + n * Q + Q],
                            ps[:, 0:Q], AF.Relu,
                            bias=f1b[:, mt:mt + 1], scale=1.0)
                    f2h = wpool.tile([128, 4 * D], bf16, tag='f2h', bufs=2,
                                     name=f'f2h_{l}_{n}_{half}')
                    for ktl in range(4):
                        kt = half * 4 + ktl
                        nc.sync.dma_start(f2h[:, ktl * D:(ktl + 1) * D],
                                          A['f2_bf'][l][kt * 128:(kt + 1) * 128, :])
                    for mt in range(2):
                        ps = pmm.tile([128, 512], f32, tag='mm', bufs=3,
                                      name='ps_f2')
                        for ktl in range(4):
                            nc.tensor.matmul(
                                ps[:, 0:Q],
                                f2h[:, ktl * D + mt * 128: ktl * D + mt * 128 + 128],
                                h1h[:, ktl * TOK + n * Q: ktl * TOK + n * Q + Q],
                                start=(ktl == 0), stop=(ktl == 3))
                        sl = slice(mt * TOK + n * Q, mt * TOK + n * Q + Q)
                        if half == 0:
                            nc.vector.scalar_tensor_tensor(
                                z3[:, sl], ps[:, 0:Q], f2b[:, mt:mt + 1],
                                out_ln2[:, sl], ALU.add, ALU.add)
                        else:
                            nc.vector.tensor_add(z3[:, sl], z3[:, sl],
                                                 ps[:, 0:Q])
                layer_norm(out_new, z3, st['n3g'], st['n3b'], 'tC', 'tB', n)

            return phase_A, phase_G, phase_B, phase_G_pre

        cur[0] = out_cur
        phs = [make_phases(l) for l in range(NL)]
        phs[0][0](0)
        phs[0][0](1)
        for l in range(NL):
            pA_next = phs[l + 1][0] if l + 1 < NL else None
            phs[l][1](0)
            phs[l][3](1)
            phs[l][2](0)
            if pA_next is not None:
                pA_next(0)
            phs[l][1](1)
            phs[l][2](1)
            if pA_next is not None:
                pA_next(1)
        out_cur = cur[0]

        for mt in range(2):
            nc.sync.dma_start(out_dram[mt * 128:(mt + 1) * 128, :],
                              out_cur[:, mt * TOK:(mt + 1) * TOK].bitcast(f32))


_PROGRAM = None


def _get_program():
    global _PROGRAM
    if _PROGRAM is None:
        _PROGRAM = _build_program()
    return _PROGRAM


def kernel(**inputs):
    nc = _get_program()
    consts = _host_consts()
    weights = _host_weights(inputs)
    shared = {**consts, **weights}
    in_maps = []
    for c in range(NCORES):
        m = dict(shared)
        m.update(_host_core_inputs(inputs, c))
        in_maps.append({k: np.ascontiguousarray(v) for k, v in m.items()})
    res = bass_utils.run_bass_kernel_spmd(nc, in_maps, core_ids=list(range(NCORES)))
    outs = []
    for c in range(NCORES):
        ot = res.results[c]['out_t']          # [256, 608]
        for n in range(NB):
            outs.append(ot[:, n * Q:n * Q + LQ].T)
    return np.stack(outs, 0).astype(np.float32)


if __name__ == '__main__':
    ref = np.load('/root/problem/ref.npz')
    inp = {k: ref[k] for k in ref.files if k != '__out__'}
    out = kernel(**inp)
    exp = ref['__out__']
    err = np.abs(out - exp)
    print("max abs err:", err.max(), "scale:", np.abs(exp).std())

